# revision 1
# baseline (speedup 1.0000x reference)
"""AsyNonLocal2D (embedded-gaussian non-local attention) on 8 trn2 NeuronCores.

Sharding: core c = (batch b = c//2, query-half h = c%2). Each core computes the
full attention for 2048 query positions of one image against all 4096 reference
positions. No collectives; host slices inputs / concatenates outputs (plus
dtype/layout marshalling: weight transposes, bf16/fp8 casts, bg folded into
bo' = bo + Wo@bg since softmax rows sum to 1).

Per-core dataflow (attention matmuls bf16, phi/g projections fp8e4m3 with
DoubleRow K=256, everything accumulated in fp32 PSUM; residual path fp32):
  theta = Wt @ q + bt            [128, 2048]  bf16 operands
  phi   = Wp @ r + bp            [128, 4096]  fp8 DoubleRow, streamed per
  g     = Wg @ r                 [128, 4096]  1024-column group under the DMAs
  gT    = g^T via PE transpose (identity matmul), batched 4 tiles/bank
  attention as two q-major passes (qh = 0, 1), each over all 32 k-tiles:
     sT  = phi_kt^T @ theta[qh]            [128, 1024] PSUM fp32 (double-buffered)
     E   = exp(sT / sqrt(128))             [128, 1024] bf16 SBUF (ACT, fused scale)
     P  += E                               (DVE bf16 partial row-sums, kt<31)
     yuT += gT_kt^T @ E                    [128, 1024] PSUM fp32 accumulation
  pass A interleaves the cg1..3 projections between its k-tiles; pass A's
  softmax finale + output projection run under pass B. PSUM budget: yuT 2
  banks + sT 2x2 banks + 2 spare banks rotating projections/finale/outproj.
  rb   = allones^T @ P + allones^T @ E31   (rowsum; the all-ones stationary
                                            broadcasts it to all partitions)
  yT   = yuT * reciprocal(rb)    (128-lane DVE reciprocal, no gpsimd needed)
  out  = Wo @ yT + bo' + q       [256, 2048] fp32 + residual (fused DVE
                                  scalar_tensor_tensor)
"""

import math

import ml_dtypes
import numpy as np

import concourse.bass as bass
import concourse.mybir as mybir
import concourse.tile as tile
from concourse.bass import ts

F32 = mybir.dt.float32
BF16 = mybir.dt.bfloat16
F8 = mybir.dt.float8e4

B, CQ, CR, H, W = 4, 256, 512, 64, 64
HW = H * W          # 4096 reference positions
HALF = HW // 2      # 2048 query positions per core
QH = HALF // 2      # 1024-wide q pass
NKT = HW // 128     # 32 k tiles
NCG = 4             # 1024-wide k column groups
SCALE = 1.0 / math.sqrt(128.0)
N_CORES = 8

# packed bf16 weight blob layout (columns): wtT[2*128] woT[256] ident[128]
_WB_COLS = 2 * 128 + 256 + 128
_OFF_WT = 0
_OFF_WO = 256
_OFF_ID = 512

ADD = mybir.AluOpType.add
EXP = mybir.ActivationFunctionType.Exp


def _proj_colgroup_chunks(nc, cg, pools, sb):
    """phi/g/gT for ref columns [cg*1024, (cg+1)*1024) through the 2 spare banks,
    as a list of emission chunks to interleave between attention k-tiles.

    ref_sb column layout is cg-major: col = cg*4096 + c*1024 + j."""
    (spare,) = pools
    wpT_sb, wgT_sb, id_sb, bp_sb, ref_sb, phi_sb, g_sb, gT_sb = sb
    base = cg * 1024
    state = {}

    def rhs_pair(cp, half):
        # fp8 ref pair (Cr chunks 2cp, 2cp+1) as a DoubleRow 3D AP [128, 2, 512]
        o = cg * 4096 + cp * 2048
        return ref_sb[:, o : o + 2048].rearrange("p (k n) -> p k n", k=2)[
            :, :, half * 512 : (half + 1) * 512
        ]

    def proj_mm(which, wT, cp):
        def emit():
            if cp == 0:
                state[which] = spare.tile(
                    [128, 1024], F32, tag="spare", name=f"pj_{which}_{cg}"
                )
            p = state[which]
            lhsT = wT[:, cp * 256 : (cp + 1) * 256].rearrange("p (k m) -> p k m", k=2)
            for half in range(2):
                nc.tensor.matmul(
                    p[:, ts(half, 512)],
                    lhsT,
                    rhs_pair(cp, half),
                    start=(cp == 0),
                    stop=(cp == 1),
                    perf_mode=mybir.MatmulPerfMode.DoubleRow,
                    skip_group_check=True,
                )
        return emit

    def phi_evac():
        nc.vector.tensor_scalar_add(phi_sb[:, base : base + 1024], state["phi"][:], bp_sb)

    def g_evac():
        nc.vector.tensor_copy(g_sb[:, base : base + 1024], state["g"][:])

    def trans(half):
        def emit():
            tps = spare.tile([128, 512], BF16, tag="spare", name=f"tps_{cg}_{half}")
            for j in range(4):
                t = cg * 8 + half * 4 + j
                nc.tensor.transpose(tps[:, ts(j, 128)], g_sb[:, ts(t, 128)], id_sb)
            nc.vector.tensor_copy(
                gT_sb[:, base + half * 512 : base + (half + 1) * 512], tps[:]
            )
        return emit

    chunks = [proj_mm("phi", wpT_sb, cp) for cp in range(2)]
    chunks.append(phi_evac)
    chunks += [proj_mm("g", wgT_sb, cp) for cp in range(2)]
    chunks.append(g_evac)
    chunks += [trans(0), trans(1)]
    return chunks


def _body(tc: tile.TileContext, io: dict):
    nc = tc.nc
    q32, wb, bb, out = (io[k] for k in ("q32", "wb", "bb", "out"))

    with (
        tc.tile_pool(name="const", bufs=1) as const,
        tc.tile_pool(name="big", bufs=1) as big,
    ):
        # ---- weights / constants (packed blobs) ----
        wb_sb = const.tile([128, _WB_COLS], BF16, tag="wb")
        bb_sb = const.tile([128, 4], F32, tag="bb")  # bt | bp | bo'_0 | bo'_1
        wtT_sb = wb_sb[:, _OFF_WT : _OFF_WT + 256]
        woT_sb = wb_sb[:, _OFF_WO : _OFF_WO + 256]
        id_sb = wb_sb[:, _OFF_ID : _OFF_ID + 128]
        bt_sb = bb_sb[:, 0:1]
        bp_sb = bb_sb[:, 1:2]
        w8_sb = const.tile([128, 1024], F8, tag="w8")  # fp8 wpT | wgT pairs
        wpT8_sb = w8_sb[:, 0:512]
        wgT8_sb = w8_sb[:, 512:1024]
        ones_sb = const.tile([128, 128], BF16, tag="ones")
        nc.gpsimd.memset(ones_sb[:], 1.0)

        # ---- input DMAs (all HWDGE; qb pre-cast to bf16, refb to fp8 on host;
        # host layouts match SBUF layouts so each load is one plain 2D DMA) ----
        # order: qb (theta gate) -> refb cg0..3 (proj gates) -> qf (residual, tail)
        ref_sb = big.tile([128, 4 * HW], F8, tag="ref")
        qf_sb = big.tile([128, 2 * HALF], F32, tag="qf")
        qb_sb = big.tile([128, 2 * HALF], BF16, tag="qb")
        refb, qbv = io["refb"], io["qbv"]
        # DMA chain ordered by when each gate is needed: phi-cg0 needs
        # ref0a+w8 (+wb only via nothing), theta needs wb+qb, S(kt0) needs
        # theta+phi-cg0. qb rides the ACT HWDGE queue: it finishes ~5us in,
        # long before the first exp, so it never contends with the
        # activation stream. refb is laid out [128, 4*HW] on the host
        # (SBUF-identical) so cg1..3 load as a single DMA.
        nc.sync.dma_start(wb_sb[:], wb[:])
        for c in range(2):
            nc.scalar.dma_start(qb_sb[:, ts(c, HALF)], qbv[:, ts(c, HALF)])
        nc.sync.dma_start(ref_sb[:, 0:2048], refb[:, 0:2048])
        nc.sync.dma_start(w8_sb[:], io["w8"][:])
        nc.sync.dma_start(bb_sb[:], bb[:])
        nc.sync.dma_start(ref_sb[:, 2048:4096], refb[:, 2048:4096])
        for cg in range(1, NCG):
            nc.sync.dma_start(ref_sb[:, ts(cg, HW)], refb[:, ts(cg, HW)])
        for c in range(2):
            nc.sync.dma_start(qf_sb[:, ts(c, HALF)], q32[:, ts(c, HALF)])

        # warm the ACT exp table during the DMA head: a 1-element dummy exp
        # triggers the ~2.7us ACT_TABLE_LOAD while ACT is otherwise idle
        # (emitted after the qb DMAs so it doesn't delay them on the ACT queue)
        warm_sb = const.tile([128, 1], BF16, tag="warm")
        nc.scalar.activation(warm_sb[:], ones_sb[:, 0:1], EXP, scale=SCALE)

        # ---- theta projection (qb lands before ref-cg0; runs first on PE) ----
        theta_sb = big.tile([128, HALF], BF16, tag="theta")
        with tc.tile_pool(name="th_ps", bufs=4, space="PSUM") as tppool:
            for qc in range(HALF // 512):
                ps = tppool.tile([128, 512], F32, tag="pp")
                for c in range(2):
                    nc.tensor.matmul(
                        ps[:],
                        wtT_sb[:, ts(c, 128)],
                        qb_sb[:, c * HALF + qc * 512 : c * HALF + (qc + 1) * 512],
                        start=(c == 0),
                        stop=(c == 1),
                    )
                nc.vector.tensor_scalar_add(theta_sb[:, ts(qc, 512)], ps[:], bt_sb)

        # ---- attention (two q passes) with streamed projections ----
        phi_sb = big.tile([128, HW], BF16, tag="phi")
        g_sb = big.tile([128, HW], BF16, tag="g")
        gT_sb = big.tile([128, HW], BF16, tag="gT")
        P_sb = big.tile([128, HALF], BF16, tag="P")
        rb_sb = big.tile([128, HALF], F32, tag="rb")
        yT_sb = big.tile([128, HALF], BF16, tag="yT")
        out_sb = big.tile([128, 2 * HALF], F32, tag="outsb")

        with (
            tc.tile_pool(name="spare_ps", bufs=1, space="PSUM") as spare,
            tc.tile_pool(name="y_ps", bufs=1, space="PSUM") as ypool,
            tc.tile_pool(name="s_ps", bufs=2, space="PSUM") as spool,
            tc.tile_pool(name="E_sb", bufs=10) as epool,
        ):
            proj_pools = (spare,)
            proj_sb = (wpT8_sb, wgT8_sb, id_sb, bp_sb, ref_sb, phi_sb, g_sb, gT_sb)
            for chunk in _proj_colgroup_chunks(nc, 0, proj_pools, proj_sb):
                chunk()

            def attention_kt(kt, qh, yuT, first, last):
                # returns the E tile (the last one is row-summed directly)
                sT = spool.tile([128, QH], F32, tag="sT")
                for qc in range(2):
                    nc.tensor.matmul(
                        sT[:, ts(qc, 512)],
                        phi_sb[:, ts(kt, 128)],
                        theta_sb[:, qh * QH + qc * 512 : qh * QH + (qc + 1) * 512],
                        start=True,
                        stop=True,
                    )
                E = epool.tile([128, QH], BF16, tag="E")
                nc.scalar.activation(E[:], sT[:], EXP, scale=SCALE)
                pcol = qh * QH
                if first:
                    nc.vector.tensor_copy(P_sb[:, pcol : pcol + QH], E[:])
                elif not last:  # last tile's sum is folded into the rowsum matmul
                    nc.vector.tensor_add(
                        P_sb[:, pcol : pcol + QH], P_sb[:, pcol : pcol + QH], E[:]
                    )
                for qc in range(2):
                    nc.tensor.matmul(
                        yuT[:, ts(qc, 512)],
                        gT_sb[:, ts(kt, 128)],
                        E[:, ts(qc, 512)],
                        start=first,
                        stop=last,
                        skip_group_check=True,
                    )
                return E

            def finale(qh, yuT, E_last):
                # rowsum via all-ones stationary: every output partition gets
                # sum_k P[k, :] -- the partition broadcast is free in the matmul.
                # Pipelined at 512 granularity to shorten the serial tail.
                pcol = qh * QH
                rb_ps = spare.tile([128, QH], F32, tag="spare", name=f"rb_{qh}")
                for qc in range(2):
                    nc.tensor.matmul(
                        rb_ps[:, ts(qc, 512)],
                        ones_sb[:],
                        P_sb[:, pcol + qc * 512 : pcol + (qc + 1) * 512],
                        start=True,
                        stop=False,
                        skip_group_check=True,
                    )
                    nc.tensor.matmul(
                        rb_ps[:, ts(qc, 512)],
                        ones_sb[:],
                        E_last[:, ts(qc, 512)],
                        start=False,
                        stop=True,
                        skip_group_check=True,
                    )
                    o = pcol + qc * 512
                    nc.vector.reciprocal(rb_sb[:, o : o + 512], rb_ps[:, ts(qc, 512)])
                    nc.vector.tensor_mul(
                        yT_sb[:, o : o + 512], yuT[:, ts(qc, 512)], rb_sb[:, o : o + 512]
                    )

            def outproj(qh, pool2=None):
                # out_sb column layout: qh*2048 + oc*1024 + j  (one DMA per qh)
                pcol = qh * QH
                for oc in range(2):
                    pool = pool2 if (oc == 1 and pool2 is not None) else spare
                    ops = pool.tile(
                        [128, QH],
                        F32,
                        tag="yuT" if pool is pool2 else "spare",
                        name=f"op_{qh}_{oc}",
                    )
                    for qc in range(2):
                        nc.tensor.matmul(
                            ops[:, ts(qc, 512)],
                            woT_sb[:, ts(oc, 128)],
                            yT_sb[:, pcol + qc * 512 : pcol + (qc + 1) * 512],
                            start=True,
                            stop=True,
                        )
                    ocol = qh * HALF + oc * QH
                    nc.vector.scalar_tensor_tensor(
                        out_sb[:, ocol : ocol + QH],
                        ops[:],
                        bb_sb[:, 2 + oc : 3 + oc],
                        qf_sb[:, oc * HALF + pcol : oc * HALF + pcol + QH],
                        op0=ADD,
                        op1=ADD,
                    )
                    nc.sync.dma_start(
                        out[:, ocol : ocol + QH], out_sb[:, ocol : ocol + QH]
                    )

            # pass A (qh=0): projections for cg1..3 interleaved between k-tiles
            yuT_A = ypool.tile([128, QH], F32, tag="yuT")
            E_last = None
            deferred = []
            for cg in range(NCG):
                chunks = (
                    _proj_colgroup_chunks(nc, cg + 1, proj_pools, proj_sb)
                    if cg < NCG - 1
                    else []
                )
                # defer each group's last transpose batch into the next
                # window (its gT tiles are consumed from slot 4 onward)
                if chunks:
                    chunks, deferred = deferred + chunks[:-1], [chunks[-1]]
                else:
                    chunks, deferred = deferred, []
                ci = 0
                # window 0: cg1's ref DMA lands ~2 k-tiles in; emitting its
                # chunks earlier would stall the static PE stream on the DMA
                delay = 2 if cg == 0 else 0
                for i, kt in enumerate(range(cg * 8, cg * 8 + 8)):
                    E_last = attention_kt(
                        kt, 0, yuT_A, first=(kt == 0), last=(kt == NKT - 1)
                    )
                    while ci < len(chunks) and ci < max(0, i + 1 - delay):
                        chunks[ci]()
                        ci += 1
                while ci < len(chunks):
                    chunks[ci]()
                    ci += 1
            finale(0, yuT_A, E_last)

            # pass B (qh=1); pass A's output projection is emitted a few
            # k-tiles in so the PE stream is not stalled on yT_A at the
            # pass boundary
            yuT_B = ypool.tile([128, QH], F32, tag="yuT")
            for kt in range(NKT):
                E_last = attention_kt(
                    kt, 1, yuT_B, first=(kt == 0), last=(kt == NKT - 1)
                )
                if kt == 3:
                    outproj(0)
            finale(1, yuT_B, E_last)
            outproj(1, pool2=ypool)


def build_nc() -> bass.Bass:
    from concourse import bacc

    nc = bacc.Bacc("TRN2", target_bir_lowering=False, debug=False)
    io = {
        "q32": nc.dram_tensor("q32", [128, 2 * HALF], F32, kind="ExternalInput").ap(),
        "qbv": nc.dram_tensor("qbv", [128, 2 * HALF], BF16, kind="ExternalInput").ap(),
        "refb": nc.dram_tensor("refb", [128, 4 * HW], F8, kind="ExternalInput").ap(),
        "w8": nc.dram_tensor("w8", [128, 1024], F8, kind="ExternalInput").ap(),
        "wb": nc.dram_tensor("wb", [128, _WB_COLS], BF16, kind="ExternalInput").ap(),
        "bb": nc.dram_tensor("bb", [128, 4], F32, kind="ExternalInput").ap(),
        "out": nc.dram_tensor("out", [128, 2 * HALF], F32, kind="ExternalOutput").ap(),
    }
    with tile.TileContext(nc) as tc:
        _body(tc, io)
    nc.compile()
    return nc


def make_in_maps(query, reference, Wg, bg, Wt, bt, Wp, bp, Wo, bo):
    bf = ml_dtypes.bfloat16
    f32 = np.float32
    query = np.ascontiguousarray(np.asarray(query, f32))
    reference = np.ascontiguousarray(np.asarray(reference, f32))
    Wg, bg, Wt, bt, Wp, bp, Wo, bo = (
        np.asarray(x, f32) for x in (Wg, bg, Wt, bt, Wp, bp, Wo, bo)
    )
    wb = np.empty((128, _WB_COLS), bf)
    wb[:, _OFF_WT : _OFF_WT + 256] = (
        np.ascontiguousarray(Wt.T).reshape(2, 128, 128).transpose(1, 0, 2).reshape(128, 256).astype(bf)
    )
    wb[:, _OFF_WO : _OFF_WO + 256] = Wo.T.astype(bf)
    wb[:, _OFF_ID : _OFF_ID + 128] = np.eye(128, dtype=bf)
    bo2 = bo + Wo @ bg
    bb = np.stack([bt, bp, bo2[:128], bo2[128:]], axis=1).astype(f32)  # [128, 4]
    f8np = mybir.dt.np(F8)
    w8 = np.empty((128, 1024), f8np)
    w8[:, 0:512] = (
        np.ascontiguousarray(Wp.T).reshape(4, 128, 128).transpose(1, 0, 2).reshape(128, 512).astype(f8np)
    )
    w8[:, 512:1024] = (
        np.ascontiguousarray(Wg.T).reshape(4, 128, 128).transpose(1, 0, 2).reshape(128, 512).astype(f8np)
    )
    common = {"wb": wb, "bb": np.ascontiguousarray(bb), "w8": w8}
    in_maps = []
    for c in range(N_CORES):
        b, h = c // 2, c % 2
        # q layout matches SBUF: [p, c*2048 + n] = query[b][c*128+p, h*2048+n]
        q_sl = np.ascontiguousarray(
            query[b]
            .reshape(2, 128, HW)[:, :, h * HALF : (h + 1) * HALF]
            .transpose(1, 0, 2)
        ).reshape(128, 2 * HALF)
        # SBUF-identical fp8 ref layout:
        # refb[p, cg*4096 + c*1024 + j] = ref[b][c*128+p, cg*1024+j]
        refb = np.ascontiguousarray(
            reference[b].reshape(4, 128, NCG, 1024).transpose(1, 2, 0, 3)
        ).reshape(128, 4 * HW).astype(mybir.dt.np(F8))
        in_maps.append(
            {
                "q32": q_sl,
                "qbv": q_sl.astype(bf),
                "refb": refb,
                **common,
            }
        )
    return in_maps


LAST_RESULTS = None


def kernel(query, reference, Wg, bg, Wt, bt, Wp, bp, Wo, bo):
    global LAST_RESULTS
    from concourse.bass_utils import run_bass_kernel_spmd

    nc = build_nc()
    in_maps = make_in_maps(query, reference, Wg, bg, Wt, bt, Wp, bp, Wo, bo)
    try:
        res = run_bass_kernel_spmd(nc, in_maps, core_ids=list(range(N_CORES)))
    except ModuleNotFoundError:
        # BASS_TRACE set under axon without the NTFF hook module present
        import os

        os.environ["BASS_NEVER_TRACE"] = "1"
        res = run_bass_kernel_spmd(nc, in_maps, core_ids=list(range(N_CORES)))
    LAST_RESULTS = res
    out = np.empty((B, CQ, H, W), np.float32)
    for c in range(N_CORES):
        b, h = c // 2, c % 2
        # device layout [p, qh*2048 + oc*1024 + j] -> [oc*128+p, qh*1024+j]
        blk = (
            res.results[c]["out"]
            .reshape(128, 2, 2, QH)
            .transpose(2, 0, 1, 3)
            .reshape(CQ, HALF)
        )
        out[b].reshape(CQ, HW)[:, h * HALF : (h + 1) * HALF] = blk
    return out



# revision 37
# speedup vs baseline: 1.1113x; 1.1113x over previous
"""AsyNonLocal2D (embedded-gaussian non-local attention) on 8 trn2 NeuronCores.

Sharding: core c = (batch b = c//2, query-half h = c%2). Each core computes the
full attention for 2048 query positions of one image against all 4096 reference
positions. No collectives; host slices inputs / concatenates outputs (plus
dtype/layout marshalling: weight transposes, bf16/fp8 casts, bg folded into
bo' = bo + Wo@bg since softmax rows sum to 1, bp dropped entirely -- it only
adds a per-query-column constant to the scores, which softmax cancels).

Per-core dataflow (theta/phi projections fp8e4m3 DoubleRow; gT computed
DIRECTLY as ref^T @ Wg^T per k-tile -- fp8 DoubleRow over Cr pairs -- so there
is no g tile and no PE transposes; attention matmuls bf16; residual path
reuses the bf16 query; output returned bf16, host upcasts):
  theta = 64*(scale/2)*(Wt @ q) + bt''   [128, 2048]  (the x64 keeps the fp8
                                          Wt entries out of the subnormal
                                          range; exp un-scales via its free
                                          input affine: exp(2/64 * sT))
  phi   = Wp @ r                         [128, 4096]
  gT_t  = r_t^T @ Wg^T                   [128, 128] per k-tile, 4-tile batches
  one 64-tile emission stream (qh = gk//32, kt = gk%32) so the pass boundary
  never drains the PE queue; per tile:
     sT  = phi_kt^T @ theta[qh]          [128, 1024] PSUM fp32 (double-buffered)
     E   = exp(2/64 * sT)                [128, 1024] bf16 SBUF (ACT)
     P  += E        (DVE accumulator P1, Pool accumulator P2 every 3rd tile;
                     kt31 folded into the rowsum matmul)
     yuT += gT_kt^T @ E                  [128, 1024] PSUM fp32, emitted LAG=2
                                         tiles late so the in-order PE SEQ
                                         never blocks on an exp semaphore
                                         before the next score matmuls
  windows of 8 tiles carry interleaved chunks: phi/gT projections for the
  next column group, theta pass-B, residual precompute (r = q + bo'), pass
  A's finale + output projection (under pass B).
  finale: rb = ones^T @ P1 + ones^T @ P2 + ones^T @ E31 (partition-broadcast
  rowsum, 3-source accumulating matmul), rbinv = 1/rb; yuT evacuated
  unnormalized; out = (Wo @ yuT) * rbinv + (q + bo') -- normalizing AFTER the
  Wo projection commutes (per-column constant) and takes recip off the
  outproj matmul path.
"""

import math

import ml_dtypes
import numpy as np

import concourse.bass as bass
import concourse.mybir as mybir
import concourse.tile as tile
from concourse.bass import ts

F32 = mybir.dt.float32
BF16 = mybir.dt.bfloat16
F8 = mybir.dt.float8e4

B, CQ, CR, H, W = 4, 256, 512, 64, 64
HW = H * W          # 4096 reference positions
HALF = HW // 2      # 2048 query positions per core
QH = HALF // 2      # 1024-wide q pass
NKT = HW // 128     # 32 k tiles
NCG = 4             # 1024-wide k column groups
SCALE = 1.0 / math.sqrt(128.0)
TUP = 64.0          # fp8 Wt upscale (kept out of subnormals); exp un-scales
N_CORES = 8
LAG = 3

# fp8 weight blob layout (columns): wpT[512] wgT[512]
_W8_COLS = 1024
# fp8 header blob: wtT8[256] | raw bytes of bb f32 [128,3] (12 cols)
_HDR_COLS = 256 + 12

ADD = mybir.AluOpType.add
EXP = mybir.ActivationFunctionType.Exp
DR = mybir.MatmulPerfMode.DoubleRow


def _body(tc: tile.TileContext, io: dict):
    nc = tc.nc
    qbv, q8v, wb, hdr, w8v, refb, out = (
        io[k] for k in ("qbv", "q8", "wb", "hdr", "w8", "refb", "out"))

    with (
        tc.tile_pool(name="const", bufs=1) as const,
        tc.tile_pool(name="big", bufs=1) as big,
    ):
        # ---- constants / weights ----
        wb_sb = const.tile([128, 256], BF16, tag="wb")   # woT
        hdr_sb = const.tile([128, _HDR_COLS], F8, tag="hdr")
        wtT8_sb = hdr_sb[:, 0:256]
        bb_sb = hdr_sb[:, 256:268].bitcast(F32)          # bt'' | bo'_0 | bo'_1
        bt_sb = bb_sb[:, 0:1]
        w8_sb = const.tile([128, _W8_COLS], F8, tag="w8")
        wpT8_sb = w8_sb[:, 0:512]
        wgT8_sb = w8_sb[:, 512:1024]
        woT_sb = wb_sb[:, 0:256]
        ones_sb = const.tile([128, 128], BF16, tag="ones")
        nc.gpsimd.memset(ones_sb[:], 1.0)

        # ---- input DMAs, ordered by need: theta wants hdr+q8A, phi/gT cg0
        # want ref0+w8, theta(B) wants q8B, woT/residual q are late ----
        ref_t = [
            big.tile([128, 4096], F8, tag=f"ref{c}", name=f"ref{c}")
            for c in range(NCG)
        ]
        qb_sb = big.tile([128, 2 * HALF], BF16, tag="qb")
        q8_sb = big.tile([128, 2 * HALF], F8, tag="q8")
        q8_h = q8_sb.rearrange("p (c n) -> p c n", c=2)
        q8v_h = q8v.rearrange("p (c n) -> p c n", c=2)
        nc.sync.dma_start(hdr_sb[:], hdr[:])
        nc.scalar.dma_start(q8_h[:, :, 0:QH], q8v_h[:, :, 0:QH])
        nc.sync.dma_start(ref_t[0][:], refb[:, 0:4096])
        nc.sync.dma_start(w8_sb[:], w8v[:])
        nc.scalar.dma_start(q8_h[:, :, QH:HALF], q8v_h[:, :, QH:HALF])
        nc.sync.dma_start(ref_t[1][:], refb[:, 4096:8192])
        nc.sync.dma_start(ref_t[2][:], refb[:, 8192:12288])
        nc.sync.dma_start(ref_t[3][:], refb[:, 12288:16384])
        nc.sync.dma_start(wb_sb[:], wb[:])
        nc.sync.dma_start(qb_sb[:], qbv[:])

        # warm the ACT exp table during the DMA head (after the q8 DMAs so
        # it doesn't delay them on the ACT queue)
        warm_sb = const.tile([128, 1], BF16, tag="warm")
        nc.scalar.activation(warm_sb[:], ones_sb[:, 0:1], EXP, scale=2.0 / TUP)

        # ---- SBUF state ----
        theta_sb = big.tile([128, HALF], BF16, tag="theta")
        phi_sb = big.tile([128, HW], BF16, tag="phi")
        gT_sb = big.tile([128, HW], BF16, tag="gT")
        P1_sb = big.tile([128, HALF], BF16, tag="P1")
        P2_sb = big.tile([128, HALF], BF16, tag="P2")
        rb_sb = big.tile([128, QH], F32, tag="rb")
        yTu_sb = big.tile([128, QH], BF16, tag="yTu")
        r_sb = big.tile([128, 2 * HALF], BF16, tag="resid")
        tmp_sb = big.tile([128, HALF], BF16, tag="tmp")
        out_sb = big.tile([128, 2 * HALF], BF16, tag="outsb")

        def theta_chunk(qc, pool=None, tag="spare"):
            # one fp8 DoubleRow matmul: K=256 via c-chunk pairs
            def mm():
                p = pool if pool is not None else spare
                ps = p.tile([128, 512], F32, tag=tag, name=f"th_{qc}")
                lhsT = wtT8_sb[:, 0:256].rearrange("p (k m) -> p k m", k=2)
                rhs = q8_sb.rearrange("p (c n) -> p c n", c=2)[
                    :, :, qc * 512 : (qc + 1) * 512]
                nc.tensor.matmul(ps[:], lhsT, rhs, start=True, stop=True,
                                 perf_mode=DR, skip_group_check=True)
                theta_chunk.ps[qc] = ps
            def evac():
                nc.vector.tensor_scalar_add(
                    theta_sb[:, ts(qc, 512)], theta_chunk.ps[qc][:], bt_sb)
            return [mm, evac]
        theta_chunk.ps = {}

        # ---- theta(A) in its own scoped PSUM pool so th0/th1 don't
        # serialize through the spare buffer ----
        with tc.tile_pool(name="th_ps", bufs=2, space="PSUM") as thp:
            tha = theta_chunk(0, pool=thp, tag="th")
            thb = theta_chunk(1, pool=thp, tag="th")
            tha[0](); thb[0](); tha[1](); thb[1]()

        with (
            tc.tile_pool(name="spare_ps", bufs=1, space="PSUM") as spare,
            tc.tile_pool(name="y_ps", bufs=1, space="PSUM") as ypool,
            tc.tile_pool(name="s_ps", bufs=2, space="PSUM") as spool,
            # NOTE: bufs must be co-prime with 3 -- the Pool engine consumes
            # every 3rd E tile, and a slot-reuse distance divisible by 3
            # would make those exp tiles (and everything queued behind them
            # on the PE SEQ) wait on the slow Pool completion chain.
            tc.tile_pool(name="E_sb", bufs=13) as epool,
        ):

            def phi_chunks(cg):
                st = {}
                def mm(cp):
                    def emit():
                        if cp == 0:
                            st["phi"] = spare.tile([128, 1024], F32,
                                                   tag="spare", name=f"pj_{cg}")
                        lhsT = wpT8_sb[:, cp * 256 : (cp + 1) * 256].rearrange(
                            "p (k m) -> p k m", k=2)
                        for half in range(2):
                            rhs = ref_t[cg][:, cp * 2048 : (cp + 1) * 2048].rearrange(
                                "p (k n) -> p k n", k=2)[
                                :, :, half * 512 : (half + 1) * 512]
                            nc.tensor.matmul(
                                st["phi"][:, ts(half, 512)], lhsT, rhs,
                                start=(cp == 0), stop=(cp == 1),
                                perf_mode=DR, skip_group_check=True)
                    return emit
                def evac(lo, hi):
                    def emit():
                        nc.vector.tensor_copy(
                            phi_sb[:, cg * 1024 + lo : cg * 1024 + hi],
                            st["phi"][:, lo:hi])
                    return emit
                if cg == 0:
                    return [mm(0), mm(1), evac(0, 512), evac(512, 1024)]
                return [mm(0), mm(1), evac(0, 1024)]

            def gt_chunks(cg):
                st = {}
                def mm(b):
                    def emit():
                        gp = spare.tile([128, 512], F32, tag="spare",
                                        name=f"gt_{cg}_{b}")
                        st[b] = gp
                        for j in range(4):
                            j0 = (b * 4 + j) * 128
                            for pair in range(2):
                                lhsT = ref_t[cg][:, pair * 2048 : (pair + 1) * 2048].rearrange(
                                    "p (k n) -> p k n", k=2)[:, :, j0 : j0 + 128]
                                rhs = wgT8_sb[:, pair * 256 : (pair + 1) * 256].rearrange(
                                    "p (k m) -> p k m", k=2)
                                nc.tensor.matmul(
                                    gp[:, ts(j, 128)], lhsT, rhs,
                                    start=(pair == 0), stop=(pair == 1),
                                    perf_mode=DR, skip_group_check=True)
                    return emit
                def evac(b):
                    def emit():
                        nc.vector.tensor_copy(
                            gT_sb[:, cg * 1024 + b * 512 : cg * 1024 + (b + 1) * 512],
                            st[b][:])
                    return emit
                return [mm(0), evac(0), mm(1), evac(1)]

            def r_chunk(oc, qh):
                def emit():
                    o = oc * HALF + qh * QH
                    nc.vector.tensor_scalar_add(
                        r_sb[:, o : o + QH], qb_sb[:, o : o + QH],
                        bb_sb[:, 1 + oc : 2 + oc])
                return emit

            def score_kt(kt, qh):
                sT = spool.tile([128, QH], F32, tag="sT")
                for qc in range(2):
                    nc.tensor.matmul(
                        sT[:, ts(qc, 512)],
                        phi_sb[:, ts(kt, 128)],
                        theta_sb[:, qh * QH + qc * 512 : qh * QH + (qc + 1) * 512],
                        start=True, stop=True, skip_group_check=True)
                E = epool.tile([128, QH], BF16, tag="E")
                nc.scalar.activation(E[:], sT[:], EXP, scale=2.0 / TUP)
                return E

            def p_accum(kt, qh, E):
                if kt == NKT - 1:
                    return  # E31 folded into the rowsum matmul
                pcol = qh * QH
                if kt % 3 == 1:  # Pool takes every 3rd tile (~2us/op there)
                    dst, eng, first = P2_sb[:, pcol : pcol + QH], nc.gpsimd, kt == 1
                else:
                    dst, eng, first = P1_sb[:, pcol : pcol + QH], nc.vector, kt == 0
                if first:
                    eng.tensor_copy(dst, E[:])
                else:
                    eng.tensor_add(dst, dst, E[:])

            def yuT_kt(kt, yuT, E, first, last):
                for qc in range(2):
                    nc.tensor.matmul(
                        yuT[:, ts(qc, 512)],
                        gT_sb[:, ts(kt, 128)],
                        E[:, ts(qc, 512)],
                        start=first, stop=last, skip_group_check=True)

            fin = {}

            def finale_chunks(qh, get_yuT, get_E31):
                # rb is a 3-source accumulating matmul (P2, P1, E31): the
                # P2/P1 matmuls are emitted early (they don't touch the last
                # exp), so only the E31 matmul trails it.  The yT normalize
                # multiply doubles as the yuT PSUM evacuation.
                pcol = qh * QH
                def rb_early():
                    rb_ps = spare.tile([128, QH], F32, tag="spare",
                                       name=f"rb_{qh}")
                    fin[qh] = rb_ps
                    for i, src in enumerate((P2_sb, P1_sb)):
                        nc.tensor.matmul(
                            rb_ps[:], ones_sb[:],
                            src[:, pcol : pcol + QH],
                            start=(i == 0), stop=False, skip_group_check=True)
                def rb_late():
                    nc.tensor.matmul(
                        fin[qh][:], ones_sb[:], get_E31()[:],
                        start=False, stop=True, skip_group_check=True)
                def recip(qc):
                    def emit():
                        nc.vector.reciprocal(
                            rb_sb[:, ts(qc, 512)], fin[qh][:, ts(qc, 512)])
                    return emit
                def yT(qc):
                    def emit():
                        nc.vector.tensor_mul(
                            yT_sb[:, ts(qc, 512)],
                            get_yuT()[:, ts(qc, 512)],
                            rb_sb[:, ts(qc, 512)])
                    return emit
                return [rb_early, rb_late, recip(0), yT(0), recip(1), yT(1)]

            def outproj_chunks(qh, pool2=None):
                # out = Wo @ yT + (q + bo'); one chunk per oc;
                # out_sb column layout: qh*2048 + oc*1024 + j
                def oc_chunk(oc):
                    def emit():
                        pool = pool2 if (oc == 1 and pool2 is not None) else spare
                        ops = pool.tile(
                            [128, QH], F32,
                            tag="yuT" if pool is pool2 else "spare",
                            name=f"op_{qh}_{oc}")
                        for qc in range(2):
                            nc.tensor.matmul(
                                ops[:, ts(qc, 512)],
                                woT_sb[:, ts(oc, 128)],
                                yT_sb[:, ts(qc, 512)],
                                start=True, stop=True, skip_group_check=True)
                        for qc in range(2):
                            ocol = qh * HALF + oc * QH + qc * 512
                            eng = nc.gpsimd if (qh == 1 and oc == 1) else nc.vector
                            eng.tensor_add(
                                out_sb[:, ocol : ocol + 512],
                                ops[:, ts(qc, 512)],
                                r_sb[:, oc * HALF + qh * QH + qc * 512 :
                                     oc * HALF + qh * QH + qc * 512 + 512])
                        ocol = qh * HALF + oc * QH
                        nc.sync.dma_start(
                            out[:, ocol : ocol + QH], out_sb[:, ocol : ocol + QH])
                    return emit
                return [oc_chunk(0), oc_chunk(1)]

            # ---- head: phi/gT cg0 ----
            pc0 = phi_chunks(0)
            gc0 = gt_chunks(0)
            pc0[0](); pc0[1]()          # phi cg0 mms
            pc0[2](); pc0[3]()          # phi cg0 evac halves
            gc0[0](); gc0[1](); gc0[2](); gc0[3]()

            # ---- window chunk schedules (8 windows x 8 k-tiles) ----
            yuT_t = {}
            win = [
                theta_chunk(2) + theta_chunk(3) + phi_chunks(1) + gt_chunks(1),
                [r_chunk(0, 0), r_chunk(1, 0)] + phi_chunks(2) + gt_chunks(2),
                [r_chunk(0, 1), r_chunk(1, 1)] + phi_chunks(3) + gt_chunks(3),
                [],
                finale_chunks(0, lambda: yuT_t[0], lambda: E31_t[0]),
                outproj_chunks(0),
                [],
                [],
            ]
            E31_t = {}
            pend = []
            E_last = None
            for w in range(8):
                qh0 = w // 4
                chunks = win[w]
                ci = 0
                delay = 2 if w == 0 else 0
                per_slot = 2 if w < 3 else 1
                for i in range(8):
                    gk = w * 8 + i
                    qh, kt = gk // NKT, gk % NKT
                    if kt == 0:
                        yuT_t[qh] = ypool.tile([128, QH], F32, tag="yuT",
                                               name=f"yuT_{qh}")
                    E = score_kt(kt, qh)
                    E_last = E
                    if kt == NKT - 1:
                        E31_t[qh] = E
                    p_accum(kt, qh, E)
                    pend.append((gk, E))
                    if len(pend) > LAG:
                        pgk, pE = pend.pop(0)
                        pqh, pkt = pgk // NKT, pgk % NKT
                        yuT_kt(pkt, yuT_t[pqh], pE,
                               first=(pkt == 0), last=(pkt == NKT - 1))
                    while ci < len(chunks) and ci < max(0, (i + 1 - delay) * per_slot):
                        chunks[ci]()
                        ci += 1
                while ci < len(chunks):
                    chunks[ci]()
                    ci += 1
            for pgk, pE in pend:
                pqh, pkt = pgk // NKT, pgk % NKT
                yuT_kt(pkt, yuT_t[pqh], pE,
                       first=(pkt == 0), last=(pkt == NKT - 1))

            # ---- tail: pass-B finale + output projection ----
            for ch in finale_chunks(1, lambda: yuT_t[1], lambda: E31_t[1]):
                ch()
            for ch in outproj_chunks(1, pool2=ypool):
                ch()


def build_nc() -> bass.Bass:
    from concourse import bacc

    nc = bacc.Bacc("TRN2", target_bir_lowering=False, debug=False)
    io = {
        "qbv": nc.dram_tensor("qbv", [128, 2 * HALF], BF16, kind="ExternalInput").ap(),
        "q8": nc.dram_tensor("q8", [128, 2 * HALF], F8, kind="ExternalInput").ap(),
        "refb": nc.dram_tensor("refb", [128, 4 * HW], F8, kind="ExternalInput").ap(),
        "w8": nc.dram_tensor("w8", [128, _W8_COLS], F8, kind="ExternalInput").ap(),
        "wb": nc.dram_tensor("wb", [128, 256], BF16, kind="ExternalInput").ap(),
        "hdr": nc.dram_tensor("hdr", [128, _HDR_COLS], F8, kind="ExternalInput").ap(),
        "out": nc.dram_tensor("out", [128, 2 * HALF], BF16, kind="ExternalOutput").ap(),
    }
    with tile.TileContext(nc) as tc:
        _body(tc, io)
    nc.compile()
    return nc


def make_in_maps(query, reference, Wg, bg, Wt, bt, Wp, bp, Wo, bo):
    bf = ml_dtypes.bfloat16
    f32 = np.float32
    query = np.ascontiguousarray(np.asarray(query, f32))
    reference = np.ascontiguousarray(np.asarray(reference, f32))
    Wg, bg, Wt, bt, Wp, bp, Wo, bo = (
        np.asarray(x, f32) for x in (Wg, bg, Wt, bt, Wp, bp, Wo, bo)
    )
    # fold the attention scale (and the fp8 anti-subnormal upscale TUP) into
    # the theta projection; exp un-scales via its input affine.  bp is
    # dropped (softmax-invariant).
    alpha = SCALE / 2.0 * TUP
    f8np = mybir.dt.np(F8)
    wb = np.ascontiguousarray(Wo.T.astype(bf))  # [128, 256]
    bo2 = bo + Wo @ bg
    bb = np.ascontiguousarray(
        np.stack([bt * alpha, bo2[:128], bo2[128:]], axis=1).astype(f32))
    w8 = np.empty((128, _W8_COLS), f8np)
    w8[:, 0:512] = (
        np.ascontiguousarray(Wp.T).reshape(4, 128, 128).transpose(1, 0, 2)
        .reshape(128, 512).astype(f8np)
    )
    w8[:, 512:1024] = (
        np.ascontiguousarray(Wg.T).reshape(4, 128, 128).transpose(1, 0, 2)
        .reshape(128, 512).astype(f8np)
    )
    hdr = np.empty((128, _HDR_COLS), f8np)
    hdr[:, 0:256] = (
        np.ascontiguousarray((Wt * alpha).T).reshape(2, 128, 128)
        .transpose(1, 0, 2).reshape(128, 256).astype(f8np)
    )
    hdr[:, 256:268] = bb.view(np.uint8).view(f8np)  # raw f32 bytes
    common = {"wb": wb, "hdr": hdr, "w8": w8}
    in_maps = []
    for c in range(N_CORES):
        b, h = c // 2, c % 2
        # q layout matches SBUF: [p, c*2048 + n] = query[b][c*128+p, h*2048+n]
        q_sl = np.ascontiguousarray(
            query[b]
            .reshape(2, 128, HW)[:, :, h * HALF : (h + 1) * HALF]
            .transpose(1, 0, 2)
        ).reshape(128, 2 * HALF)
        # SBUF-identical fp8 ref layout:
        # refb[p, cg*4096 + c*1024 + j] = ref[b][c*128+p, cg*1024+j]
        refb = np.ascontiguousarray(
            reference[b].reshape(4, 128, NCG, 1024).transpose(1, 2, 0, 3)
        ).reshape(128, 4 * HW).astype(f8np)
        in_maps.append({
            "qbv": q_sl.astype(bf),
            "q8": q_sl.astype(f8np),
            "refb": refb,
            **common,
        })
    return in_maps


LAST_RESULTS = None


def kernel(query, reference, Wg, bg, Wt, bt, Wp, bp, Wo, bo):
    global LAST_RESULTS
    from concourse.bass_utils import run_bass_kernel_spmd

    nc = build_nc()
    in_maps = make_in_maps(query, reference, Wg, bg, Wt, bt, Wp, bp, Wo, bo)
    try:
        res = run_bass_kernel_spmd(nc, in_maps, core_ids=list(range(N_CORES)))
    except ModuleNotFoundError:
        # BASS_TRACE set under axon without the NTFF hook module present
        import os

        os.environ["BASS_NEVER_TRACE"] = "1"
        res = run_bass_kernel_spmd(nc, in_maps, core_ids=list(range(N_CORES)))
    LAST_RESULTS = res
    out = np.empty((B, CQ, H, W), np.float32)
    for c in range(N_CORES):
        b, h = c // 2, c % 2
        # device layout [p, qh*2048 + oc*1024 + j] -> [oc*128+p, qh*1024+j]
        blk = (
            res.results[c]["out"].astype(np.float32)
            .reshape(128, 2, 2, QH)
            .transpose(2, 0, 1, 3)
            .reshape(CQ, HALF)
        )
        out[b].reshape(CQ, HW)[:, h * HALF : (h + 1) * HALF] = blk
    return out
